# revision 11
# baseline (speedup 1.0000x reference)
"""Causal self-attention (B=4, T=2048, D=1024, H=16) on 8 TRN2 NeuronCores.

Sharding: core i = (batch b = i//2, head-group g = i%2). Data parallel on B,
tensor parallel on heads (8 heads per group): qkv_proj columns and out_proj
rows split per head group. Each core computes a partial [D, T] output^T for
its batch; host sums the two group partials per batch, transposes, adds bias.

v3 design: all-bf16 PE datapath (fp32 PSUM accumulation), Q^T/K^T resident in
SBUF, and a qc-major fused schedule: attention blocks run (qc, pair) ordered so
each q-chunk level only needs transposes/projections for t <= 512*(qc+1);
setup work (x transpose, V/Q/K projections), the output projection and DMAs
are lazily interleaved between S/AV groups to keep the PE busy while the
scalar engine streams exp(). The 1/sqrt(dh) scale is folded into W_q on host.
Elementwise load is split: exp on Scalar, normalize/copies on Vector, causal
mask-mul + Q/K psum casts on GpSimd.
"""

import numpy as np
import ml_dtypes

import concourse.bacc as bacc
import concourse.tile as tile
import concourse.mybir as mybir
from concourse import bass_utils
from concourse.bass import ts

F32 = mybir.dt.float32
BF = mybir.dt.bfloat16
EXP = mybir.ActivationFunctionType.Exp

T = 2048
TT = 16          # t tiles of 128
NP = 4           # head pairs per core
NQC = 4          # q chunks of 512

_CACHE = {}
_last_in_maps = None


def _build(CT):
    """CT = number of 128-row c-tiles in the (possibly bias-augmented) x/W."""
    nc = bacc.Bacc("TRN2", target_bir_lowering=False, debug=False)
    C = CT * 128

    xa = nc.dram_tensor("xa", [T, C], BF, kind="ExternalInput").ap()
    wq = nc.dram_tensor("wq", [C, 512], BF, kind="ExternalInput").ap()
    wk = nc.dram_tensor("wk", [C, 512], BF, kind="ExternalInput").ap()
    wv = nc.dram_tensor("wv", [C, 512], BF, kind="ExternalInput").ap()
    wo = nc.dram_tensor("wo", [512, 1024], BF, kind="ExternalInput").ap()
    tri = nc.dram_tensor("tri", [128, 128], BF, kind="ExternalInput").ap()
    idn = nc.dram_tensor("idn", [128, 128], BF, kind="ExternalInput").ap()
    ot = nc.dram_tensor("ot", [1024, T], F32, kind="ExternalOutput").ap()

    with tile.TileContext(nc) as tc:
        with (
            tc.tile_pool(name="persist", bufs=1) as persist,
        ):
            mm = nc.tensor.matmul
            mmt = nc.tensor.transpose

            # resident tensors (per-partition bytes in comments)
            xT = persist.tile([128, CT, T], BF)            # 32K (CT=8)
            QK = persist.tile([128, NP, 2, NQC, 512], BF)  # 32K  [q=0/k=1]
            vS = persist.tile([128, TT, 8, 65], BF)        # 16.6K [k,tt,head,d+1]
            OT = persist.tile([128, NP, T], BF)            # 16.4K
            wq_sb = persist.tile([128, CT, NP, 128], BF)   # 8.2K
            wk_sb = persist.tile([128, CT, NP, 128], BF)   # 8.2K
            wv_sb = persist.tile([128, CT, 512], BF)       # 8.2K
            wo_sb = persist.tile([128, NP, 1024], BF)      # 8.2K
            tr = persist.tile([128, 128], BF)
            ident = persist.tile([128, 128], BF)
            nc.vector.memset(vS[:, :, :, 64:65], 1.0)
            nc.sync.dma_start(out=ident, in_=idn)

            with (
                tc.tile_pool(name="xnat", bufs=4) as xnat,
                tc.tile_pool(name="ptp", bufs=10) as ptpool,
                tc.tile_pool(name="rsm", bufs=6) as rpool,
                tc.tile_pool(name="rbcp", bufs=4) as rbcpool,
                tc.tile_pool(name="obnc", bufs=4) as opool,
                tc.tile_pool(name="psS", bufs=2, space="PSUM") as psS,      # 4 banks
                tc.tile_pool(name="psAv", bufs=2, space="PSUM") as psAv,    # 2 banks
                tc.tile_pool(name="pso", bufs=2, space="PSUM") as psO,      # 2 banks
            ):
                # ---------- setup work units (emitted lazily) ----------
                def load_weights():
                    nc.gpsimd.dma_start(out=tr, in_=tri)
                    nc.gpsimd.dma_start(
                        out=wv_sb,
                        in_=wv.rearrange("(ct P) f -> P ct f", P=128),
                    )
                    nc.gpsimd.dma_start(
                        out=wq_sb,
                        in_=wq.rearrange("(ct P) (np f) -> P ct np f", P=128, np=NP),
                    )
                    nc.gpsimd.dma_start(
                        out=wk_sb,
                        in_=wk.rearrange("(ct P) (np f) -> P ct np f", P=128, np=NP),
                    )

                def load_wo():
                    nc.gpsimd.dma_start(
                        out=wo_sb,
                        in_=wo.rearrange("(np P) f -> P np f", P=128),
                    )

                def transpose_tt(tt):
                    # one DMA per 128-token slab; transpose quads share a psum
                    # tile so one DVE copy moves 4 c-tiles into xT
                    xn = xnat.tile([128, C], BF)
                    nc.sync.dma_start(out=xn, in_=xa[ts(tt, 128), :])
                    for q4 in range(0, CT, 4):
                        ncc = min(4, CT - q4)
                        pt_ = psO.tile([128, 512], F32, name="pso", tag="pso")
                        for k in range(ncc):
                            cc = q4 + k
                            mmt(
                                pt_[:, 64 * k : 64 * k + 64].bitcast(BF),
                                xn[:, ts(cc, 128)],
                                ident,
                            )
                        nc.vector.tensor_copy(
                            out=xT[:, q4 : q4 + ncc, ts(tt, 128)],
                            in_=pt_[:, 0 : 64 * ncc]
                            .bitcast(BF)
                            .rearrange("p (c t) -> p c t", c=ncc),
                        )

                def vproj_tt(tt):
                    ps = psO.tile([128, 512], F32, name="pso", tag="pso")
                    for cc in range(CT):
                        mm(
                            ps,
                            lhsT=xT[:, cc, ts(tt, 128)],
                            rhs=wv_sb[:, cc, :],
                            start=(cc == 0),
                            stop=(cc == CT - 1),
                        )
                    nc.vector.tensor_copy(
                        out=vS[:, tt, :, 0:64],
                        in_=ps.rearrange("p (h d) -> p h d", h=8),
                    )

                def proj_chunk(p, kind, tc_):
                    # kind: 0 = q, 1 = k
                    w_sb = wq_sb if kind == 0 else wk_sb
                    ps = psO.tile([128, 512], F32, name="pso", tag="pso")
                    for cc in range(CT):
                        mm(
                            ps,
                            lhsT=w_sb[:, cc, p, :],
                            rhs=xT[:, cc, ts(tc_, 512)],
                            start=(cc == 0),
                            stop=(cc == CT - 1),
                        )
                    nc.vector.tensor_copy(out=QK[:, p, kind, tc_, :], in_=ps)

                def phase3_half(tc_, h):
                    for ft in range(4 * h, 4 * h + 4):
                        ps = psO.tile([128, 512], F32, name="pso", tag="pso")
                        for p in range(NP):
                            mm(
                                ps,
                                lhsT=wo_sb[:, p, ts(ft, 128)],
                                rhs=OT[:, p, ts(tc_, 512)],
                                start=(p == 0),
                                stop=(p == NP - 1),
                            )
                        ob = opool.tile([128, 512], F32)
                        nc.vector.tensor_copy(out=ob, in_=ps)
                        nc.sync.dma_start(out=ot[ts(ft, 128), ts(tc_, 512)], in_=ob)

                # lazy emission bookkeeping
                done_tt = [0]
                done_proj = set()
                fillers = []

                def need_tt(up_to):
                    up_to = min(up_to, TT)
                    lo = done_tt[0]
                    for tt in range(lo, up_to):
                        transpose_tt(tt)
                    for tt in range(lo, up_to):
                        vproj_tt(tt)
                    done_tt[0] = max(done_tt[0], up_to)

                def need_proj(p, kind, tc_):
                    if p >= NP or tc_ >= NQC:
                        return
                    key = (p, kind, tc_)
                    if key in done_proj:
                        return
                    done_proj.add(key)
                    proj_chunk(p, kind, tc_)

                def pop_filler():
                    if fillers:
                        fillers.pop(0)()

                # ---------- attention work ----------
                qch = lambda p, tc_: QK[:, p, 0, tc_, :]
                kch = lambda p, tc_: QK[:, p, 1, tc_, :]
                avs = {}
                pts = {}
                level_done = [0] * NQC

                def s_exp(p, qc, j):
                    off = max(0, 128 * j - 512 * qc)
                    sg = psS.tile([128, 2, 512], F32)
                    kc = kch(p, j // 4)
                    qc_t = qch(p, qc)
                    jo = 128 * (j % 4)
                    for m in range(2):
                        mm(
                            sg[:, m, off:],
                            lhsT=kc[64 * m : 64 * m + 64, jo : jo + 128],
                            rhs=qc_t[64 * m : 64 * m + 64, off:],
                            start=True,
                            stop=True,
                        )
                    ptile = ptpool.tile([128, 2, 512], BF)
                    nc.scalar.activation(
                        out=ptile[:, :, off:], in_=sg[:, :, off:], func=EXP
                    )
                    if j >= 4 * qc:
                        nc.gpsimd.tensor_mul(
                            ptile[:, :, off : off + 128],
                            ptile[:, :, off : off + 128],
                            tr[:, None, :].to_broadcast([128, 2, 128]),
                        )
                    pts[(p, qc, j)] = (ptile, off)

                def av_mm(p, qc, j, nj):
                    ptile, off = pts.pop((p, qc, j))
                    av = avs[(p, qc)]
                    for m in range(2):
                        mm(
                            av[m][:65, off:],
                            lhsT=vS[:, j, 2 * p + m, :],
                            rhs=ptile[:, m, off:],
                            start=(j == 0),
                            stop=(j == nj - 1),
                        )

                def normalize(p, qc):
                    av = avs.pop((p, qc))
                    rsbs = []
                    for m in range(2):
                        rsb = rpool.tile([1, 512], F32, name="rsb", tag="rsb")
                        nc.vector.tensor_copy(out=rsb, in_=av[m][64:65, :])
                        # unnormalized O~ out of PSUM so the av bank frees fast
                        nc.vector.tensor_copy(
                            out=OT[64 * m : 64 * m + 64, p, ts(qc, 512)],
                            in_=av[m][0:64, :],
                        )
                        rsbs.append(rsb)
                    for m in range(2):
                        rinv = rpool.tile([1, 512], F32, name="rinv", tag="rinv")
                        nc.vector.reciprocal_approx_fast(out=rinv, in_=rsbs[m])
                        rinv_b = rpool.tile([1, 512], BF, name="rinvb", tag="rinvb")
                        nc.vector.tensor_copy(out=rinv_b, in_=rinv)
                        rb = rbcpool.tile([128, 512], BF, name="rb", tag="rb")
                        nc.gpsimd.partition_broadcast(rb, rinv_b)
                        sl = OT[64 * m : 64 * m + 64, p, ts(qc, 512)]
                        nc.vector.tensor_mul(sl, sl, rb[64 * m : 64 * m + 64, :])
                    level_done[qc] += 1
                    if level_done[qc] == NP:
                        # all pairs finished this t-chunk: output projection
                        fillers.insert(0, lambda a=qc: phase3_half(a, 0))
                        fillers.insert(1, lambda a=qc: phase3_half(a, 1))

                # groups: qc-major so each level only needs setup through
                # t = 512*(qc+1); pairs within a level share the causal shape
                groups = []
                for qc in range(NQC):
                    for p in range(NP):
                        nj = 4 * qc + 4
                        js = list(range(nj))
                        sub = [js[i : i + 3] for i in range(0, nj, 3)]
                        for gi, jg in enumerate(sub):
                            groups.append(
                                (p, qc, nj, jg, gi == 0, gi == len(sub) - 1)
                            )

                def av_group(gi):
                    p, qc, nj, jg, first, last = groups[gi]
                    if first:
                        avs[(p, qc)] = [
                            psAv.tile([128, 512], F32, name="av", tag="av")
                            for _ in range(2)
                        ]
                    for j in jg:
                        av_mm(p, qc, j, nj)
                    if last:
                        normalize(p, qc)

                # prologue: weights first (gpsimd queue, parallel to x slabs)
                load_weights()
                load_wo()
                need_tt(4)
                need_proj(0, 1, 0)
                need_proj(0, 0, 0)

                LAG = 2
                for i in range(len(groups) + LAG):
                    if i < len(groups):
                        p, qc, nj, jg, first, last = groups[i]
                        if first:
                            # hard deps for this block (usually no-ops)
                            need_tt(4 * qc + 4)
                            need_proj(p, 1, qc)
                            need_proj(p, 0, qc)
                            # soft prefetches, spread between groups
                            if p == 0:
                                fillers.append(lambda u=4 * qc + 8: need_tt(u))
                            fillers.append(
                                lambda b=p, a=qc + 1: (
                                    need_proj(b, 1, a),
                                    need_proj(b, 0, a),
                                )
                            )
                        for j in jg:
                            s_exp(p, qc, j)
                        pop_filler()
                    if i >= LAG and (i - LAG) % 2 == 1:
                        av_group(i - LAG - 1)
                        av_group(i - LAG)
                        pop_filler()
                if len(groups) % 2 == 1:
                    av_group(len(groups) - 1)
                while fillers:
                    pop_filler()

    nc.compile()
    return nc


def kernel(x, W_qkv, b_qkv, W_out, b_out):
    global _last_in_maps
    x = np.asarray(x, dtype=np.float32)
    W_qkv = np.asarray(W_qkv, dtype=np.float32)
    b_qkv = np.asarray(b_qkv, dtype=np.float32)
    W_out = np.asarray(W_out, dtype=np.float32)
    b_out = np.asarray(b_out, dtype=np.float32)
    B = x.shape[0]
    BF_NP = ml_dtypes.bfloat16

    aug = bool(np.any(b_qkv))
    CT = 9 if aug else 8
    if CT not in _CACHE:
        _CACHE[CT] = _build(CT)
    nc = _CACHE[CT]

    # triangle keep-mask for the diagonal 128 block: [p, c] = 1 if c >= p
    tri = (np.arange(128)[None, :] >= np.arange(128)[:, None]).astype(BF_NP)
    SCALE = 1.0 / np.sqrt(64.0)

    in_maps = []
    for core in range(8):
        b, g = core // 2, core % 2
        xa = x[b]
        if aug:
            pad = np.zeros((T, 128), np.float32)
            pad[:, 0] = 1.0
            xa = np.concatenate([xa, pad], axis=1)

        def wslice(col0, scale=1.0):
            w = W_qkv[:, col0 + 512 * g : col0 + 512 * g + 512]
            if aug:
                extra = np.zeros((128, 512), np.float32)
                extra[0] = b_qkv[col0 + 512 * g : col0 + 512 * g + 512]
                w = np.concatenate([w, extra], axis=0)
            return np.ascontiguousarray(w * scale).astype(BF_NP)

        in_maps.append(
            {
                "xa": np.ascontiguousarray(xa).astype(BF_NP),
                "wq": wslice(0, SCALE),
                "wk": wslice(1024),
                "wv": wslice(2048),
                "wo": np.ascontiguousarray(
                    W_out[512 * g : 512 * g + 512, :]
                ).astype(BF_NP),
                "tri": tri,
                "idn": np.eye(128, dtype=BF_NP),
            }
        )

    _last_in_maps = in_maps
    res = bass_utils.run_bass_kernel_spmd(nc, in_maps, list(range(8))).results
    out = np.empty((B, T, 1024), np.float32)
    for b in range(B):
        acc = res[2 * b]["ot"].astype(np.float32) + res[2 * b + 1]["ot"].astype(
            np.float32
        )
        out[b] = acc.T + b_out[None, :]
    return out


# revision 12
# speedup vs baseline: 1.4892x; 1.4892x over previous
"""Causal self-attention (B=4, T=2048, D=1024, H=16) on 8 TRN2 NeuronCores.

Sharding: core i = (batch b = i//2, head-group g = i%2). Data parallel on B,
tensor parallel on heads (8 heads per group): qkv_proj columns and out_proj
rows split per head group. Each core computes a partial [D, T] output^T for
its batch; host sums the two group partials per batch, transposes, adds bias.

v3 design: all-bf16 PE datapath (fp32 PSUM accumulation), Q^T/K^T resident in
SBUF, and a qc-major fused schedule: attention blocks run (qc, pair) ordered so
each q-chunk level only needs transposes/projections for t <= 512*(qc+1);
setup work (x transpose, V/Q/K projections), the output projection and DMAs
are lazily interleaved between S/AV groups to keep the PE busy while the
scalar engine streams exp(). The 1/sqrt(dh) scale is folded into W_q on host.
Elementwise load is split: exp on Scalar, normalize/copies on Vector, causal
mask-mul + Q/K psum casts on GpSimd.
"""

import numpy as np
import ml_dtypes

import concourse.bacc as bacc
import concourse.tile as tile
import concourse.mybir as mybir
from concourse import bass_utils
from concourse.bass import ts

F32 = mybir.dt.float32
BF = mybir.dt.bfloat16
EXP = mybir.ActivationFunctionType.Exp

T = 2048
TT = 16          # t tiles of 128
NP = 4           # head pairs per core
NQC = 4          # q chunks of 512

_CACHE = {}
_last_in_maps = None


def _build(CT):
    """CT = number of 128-row c-tiles in the (possibly bias-augmented) x/W."""
    nc = bacc.Bacc("TRN2", target_bir_lowering=False, debug=False)
    C = CT * 128

    xa = nc.dram_tensor("xa", [T, C], BF, kind="ExternalInput").ap()
    wq = nc.dram_tensor("wq", [C, 512], BF, kind="ExternalInput").ap()
    wk = nc.dram_tensor("wk", [C, 512], BF, kind="ExternalInput").ap()
    wv = nc.dram_tensor("wv", [C, 512], BF, kind="ExternalInput").ap()
    wo = nc.dram_tensor("wo", [512, 1024], BF, kind="ExternalInput").ap()
    tri = nc.dram_tensor("tri", [128, 128], BF, kind="ExternalInput").ap()
    idn = nc.dram_tensor("idn", [128, 128], BF, kind="ExternalInput").ap()
    ot = nc.dram_tensor("ot", [1024, T], F32, kind="ExternalOutput").ap()

    with tile.TileContext(nc) as tc:
        with (
            tc.tile_pool(name="persist", bufs=1) as persist,
        ):
            mm = nc.tensor.matmul
            mmt = nc.tensor.transpose

            # resident tensors (per-partition bytes in comments)
            xT = persist.tile([128, CT, T], BF)            # 32K (CT=8)
            QK = persist.tile([128, NP, 2, NQC, 512], BF)  # 32K  [q=0/k=1]
            vS = persist.tile([128, TT, 8, 65], BF)        # 16.6K [k,tt,head,d+1]
            OT = persist.tile([128, NP, T], BF)            # 16.4K
            wq_sb = persist.tile([128, CT, NP, 128], BF)   # 8.2K
            wk_sb = persist.tile([128, CT, NP, 128], BF)   # 8.2K
            wv_sb = persist.tile([128, CT, 512], BF)       # 8.2K
            wo_sb = persist.tile([128, NP, 1024], BF)      # 8.2K
            tr = persist.tile([128, 128], BF)
            ident = persist.tile([128, 128], BF)
            nc.vector.memset(vS[:, :, :, 64:65], 1.0)
            nc.sync.dma_start(out=ident, in_=idn)

            with (
                tc.tile_pool(name="xnat", bufs=4) as xnat,
                tc.tile_pool(name="ptp", bufs=10) as ptpool,
                tc.tile_pool(name="rsm", bufs=6) as rpool,
                tc.tile_pool(name="rbcp", bufs=4) as rbcpool,
                tc.tile_pool(name="obnc", bufs=4) as opool,
                tc.tile_pool(name="psS", bufs=2, space="PSUM") as psS,      # 4 banks
                tc.tile_pool(name="psAv", bufs=2, space="PSUM") as psAv,    # 2 banks
                tc.tile_pool(name="pso", bufs=2, space="PSUM") as psO,      # 2 banks
            ):
                # ---------- setup work units (emitted lazily) ----------
                def load_weights():
                    nc.gpsimd.dma_start(out=tr, in_=tri)
                    nc.gpsimd.dma_start(
                        out=wv_sb,
                        in_=wv.rearrange("(ct P) f -> P ct f", P=128),
                    )
                    nc.gpsimd.dma_start(
                        out=wq_sb,
                        in_=wq.rearrange("(ct P) (np f) -> P ct np f", P=128, np=NP),
                    )
                    nc.gpsimd.dma_start(
                        out=wk_sb,
                        in_=wk.rearrange("(ct P) (np f) -> P ct np f", P=128, np=NP),
                    )

                def load_wo():
                    nc.gpsimd.dma_start(
                        out=wo_sb,
                        in_=wo.rearrange("(np P) f -> P np f", P=128),
                    )

                def transpose_tt(tt):
                    # one DMA per 128-token slab; transpose quads share a psum
                    # tile so one DVE copy moves 4 c-tiles into xT
                    xn = xnat.tile([128, C], BF)
                    nc.sync.dma_start(out=xn, in_=xa[ts(tt, 128), :])
                    for q4 in range(0, CT, 4):
                        ncc = min(4, CT - q4)
                        pt_ = psO.tile([128, 512], F32, name="pso", tag="pso")
                        for k in range(ncc):
                            cc = q4 + k
                            mmt(
                                pt_[:, 64 * k : 64 * k + 64].bitcast(BF),
                                xn[:, ts(cc, 128)],
                                ident,
                            )
                        nc.vector.tensor_copy(
                            out=xT[:, q4 : q4 + ncc, ts(tt, 128)],
                            in_=pt_[:, 0 : 64 * ncc]
                            .bitcast(BF)
                            .rearrange("p (c t) -> p c t", c=ncc),
                        )

                def vproj_tt(tt):
                    ps = psO.tile([128, 512], F32, name="pso", tag="pso")
                    for cc in range(CT):
                        mm(
                            ps,
                            lhsT=xT[:, cc, ts(tt, 128)],
                            rhs=wv_sb[:, cc, :],
                            start=(cc == 0),
                            stop=(cc == CT - 1),
                        )
                    nc.vector.tensor_copy(
                        out=vS[:, tt, :, 0:64],
                        in_=ps.rearrange("p (h d) -> p h d", h=8),
                    )

                def proj_chunk(p, kind, tc_):
                    # kind: 0 = q, 1 = k
                    w_sb = wq_sb if kind == 0 else wk_sb
                    ps = psO.tile([128, 512], F32, name="pso", tag="pso")
                    for cc in range(CT):
                        mm(
                            ps,
                            lhsT=w_sb[:, cc, p, :],
                            rhs=xT[:, cc, ts(tc_, 512)],
                            start=(cc == 0),
                            stop=(cc == CT - 1),
                        )
                    nc.vector.tensor_copy(out=QK[:, p, kind, tc_, :], in_=ps)

                def phase3_half(tc_, h):
                    for ft in range(4 * h, 4 * h + 4):
                        ps = psO.tile([128, 512], F32, name="pso", tag="pso")
                        for p in range(NP):
                            mm(
                                ps,
                                lhsT=wo_sb[:, p, ts(ft, 128)],
                                rhs=OT[:, p, ts(tc_, 512)],
                                start=(p == 0),
                                stop=(p == NP - 1),
                            )
                        ob = opool.tile([128, 512], F32)
                        nc.vector.tensor_copy(out=ob, in_=ps)
                        nc.sync.dma_start(out=ot[ts(ft, 128), ts(tc_, 512)], in_=ob)

                # lazy emission bookkeeping
                done_tt = [0]
                done_proj = set()
                fillers = []

                def need_tt(up_to):
                    up_to = min(up_to, TT)
                    lo = done_tt[0]
                    for tt in range(lo, up_to):
                        transpose_tt(tt)
                    for tt in range(lo, up_to):
                        vproj_tt(tt)
                    done_tt[0] = max(done_tt[0], up_to)

                def need_proj(p, kind, tc_):
                    if p >= NP or tc_ >= NQC:
                        return
                    key = (p, kind, tc_)
                    if key in done_proj:
                        return
                    done_proj.add(key)
                    proj_chunk(p, kind, tc_)

                def pop_filler():
                    if fillers:
                        fillers.pop(0)()

                # ---------- attention work ----------
                qch = lambda p, tc_: QK[:, p, 0, tc_, :]
                kch = lambda p, tc_: QK[:, p, 1, tc_, :]
                avs = {}
                pts = {}
                level_done = [0] * NQC

                def s_exp(p, qc, j):
                    off = max(0, 128 * j - 512 * qc)
                    sg = psS.tile([128, 2, 512], F32)
                    kc = kch(p, j // 4)
                    qc_t = qch(p, qc)
                    jo = 128 * (j % 4)
                    for m in range(2):
                        mm(
                            sg[:, m, off:],
                            lhsT=kc[64 * m : 64 * m + 64, jo : jo + 128],
                            rhs=qc_t[64 * m : 64 * m + 64, off:],
                            start=True,
                            stop=True,
                        )
                    ptile = ptpool.tile([128, 2, 512], BF)
                    nc.scalar.activation(
                        out=ptile[:, :, off:], in_=sg[:, :, off:], func=EXP
                    )
                    if j >= 4 * qc:
                        nc.vector.tensor_mul(
                            ptile[:, :, off : off + 128],
                            ptile[:, :, off : off + 128],
                            tr[:, None, :].to_broadcast([128, 2, 128]),
                        )
                    pts[(p, qc, j)] = (ptile, off)

                def av_mm(p, qc, j, nj):
                    ptile, off = pts.pop((p, qc, j))
                    av = avs[(p, qc)]
                    for m in range(2):
                        mm(
                            av[m][:65, off:],
                            lhsT=vS[:, j, 2 * p + m, :],
                            rhs=ptile[:, m, off:],
                            start=(j == 0),
                            stop=(j == nj - 1),
                        )

                def normalize(p, qc):
                    av = avs.pop((p, qc))
                    rsbs = []
                    for m in range(2):
                        rsb = rpool.tile([1, 512], F32, name="rsb", tag="rsb")
                        nc.vector.tensor_copy(out=rsb, in_=av[m][64:65, :])
                        # unnormalized O~ out of PSUM so the av bank frees fast
                        nc.vector.tensor_copy(
                            out=OT[64 * m : 64 * m + 64, p, ts(qc, 512)],
                            in_=av[m][0:64, :],
                        )
                        rsbs.append(rsb)
                    for m in range(2):
                        rinv = rpool.tile([1, 512], F32, name="rinv", tag="rinv")
                        nc.vector.reciprocal_approx_fast(out=rinv, in_=rsbs[m])
                        rinv_b = rpool.tile([1, 512], BF, name="rinvb", tag="rinvb")
                        nc.vector.tensor_copy(out=rinv_b, in_=rinv)
                        rb = rbcpool.tile([128, 512], BF, name="rb", tag="rb")
                        nc.gpsimd.partition_broadcast(rb, rinv_b)
                        sl = OT[64 * m : 64 * m + 64, p, ts(qc, 512)]
                        nc.vector.tensor_mul(sl, sl, rb[64 * m : 64 * m + 64, :])
                    level_done[qc] += 1
                    if level_done[qc] == NP:
                        # all pairs finished this t-chunk: output projection
                        fillers.insert(0, lambda a=qc: phase3_half(a, 0))
                        fillers.insert(1, lambda a=qc: phase3_half(a, 1))

                # groups: qc-major so each level only needs setup through
                # t = 512*(qc+1); pairs within a level share the causal shape
                groups = []
                for qc in range(NQC):
                    for p in range(NP):
                        nj = 4 * qc + 4
                        js = list(range(nj))
                        sub = [js[i : i + 3] for i in range(0, nj, 3)]
                        for gi, jg in enumerate(sub):
                            groups.append(
                                (p, qc, nj, jg, gi == 0, gi == len(sub) - 1)
                            )

                def av_group(gi):
                    p, qc, nj, jg, first, last = groups[gi]
                    if first:
                        avs[(p, qc)] = [
                            psAv.tile([128, 512], F32, name="av", tag="av")
                            for _ in range(2)
                        ]
                    for j in jg:
                        av_mm(p, qc, j, nj)
                    if last:
                        normalize(p, qc)

                # prologue: weights first (gpsimd queue, parallel to x slabs)
                load_weights()
                load_wo()
                need_tt(4)
                need_proj(0, 1, 0)
                need_proj(0, 0, 0)

                LAG = 2
                for i in range(len(groups) + LAG):
                    if i < len(groups):
                        p, qc, nj, jg, first, last = groups[i]
                        if first:
                            # hard deps for this block (usually no-ops)
                            need_tt(4 * qc + 4)
                            need_proj(p, 1, qc)
                            need_proj(p, 0, qc)
                            # soft prefetches, spread between groups
                            if p == 0:
                                fillers.append(lambda u=4 * qc + 8: need_tt(u))
                            fillers.append(
                                lambda b=p, a=qc + 1: (
                                    need_proj(b, 1, a),
                                    need_proj(b, 0, a),
                                )
                            )
                        for j in jg:
                            s_exp(p, qc, j)
                        pop_filler()
                    if i >= LAG and (i - LAG) % 2 == 1:
                        av_group(i - LAG - 1)
                        av_group(i - LAG)
                        pop_filler()
                if len(groups) % 2 == 1:
                    av_group(len(groups) - 1)
                while fillers:
                    pop_filler()

    nc.compile()
    return nc


def kernel(x, W_qkv, b_qkv, W_out, b_out):
    global _last_in_maps
    x = np.asarray(x, dtype=np.float32)
    W_qkv = np.asarray(W_qkv, dtype=np.float32)
    b_qkv = np.asarray(b_qkv, dtype=np.float32)
    W_out = np.asarray(W_out, dtype=np.float32)
    b_out = np.asarray(b_out, dtype=np.float32)
    B = x.shape[0]
    BF_NP = ml_dtypes.bfloat16

    aug = bool(np.any(b_qkv))
    CT = 9 if aug else 8
    if CT not in _CACHE:
        _CACHE[CT] = _build(CT)
    nc = _CACHE[CT]

    # triangle keep-mask for the diagonal 128 block: [p, c] = 1 if c >= p
    tri = (np.arange(128)[None, :] >= np.arange(128)[:, None]).astype(BF_NP)
    SCALE = 1.0 / np.sqrt(64.0)

    in_maps = []
    for core in range(8):
        b, g = core // 2, core % 2
        xa = x[b]
        if aug:
            pad = np.zeros((T, 128), np.float32)
            pad[:, 0] = 1.0
            xa = np.concatenate([xa, pad], axis=1)

        def wslice(col0, scale=1.0):
            w = W_qkv[:, col0 + 512 * g : col0 + 512 * g + 512]
            if aug:
                extra = np.zeros((128, 512), np.float32)
                extra[0] = b_qkv[col0 + 512 * g : col0 + 512 * g + 512]
                w = np.concatenate([w, extra], axis=0)
            return np.ascontiguousarray(w * scale).astype(BF_NP)

        in_maps.append(
            {
                "xa": np.ascontiguousarray(xa).astype(BF_NP),
                "wq": wslice(0, SCALE),
                "wk": wslice(1024),
                "wv": wslice(2048),
                "wo": np.ascontiguousarray(
                    W_out[512 * g : 512 * g + 512, :]
                ).astype(BF_NP),
                "tri": tri,
                "idn": np.eye(128, dtype=BF_NP),
            }
        )

    _last_in_maps = in_maps
    res = bass_utils.run_bass_kernel_spmd(nc, in_maps, list(range(8))).results
    out = np.empty((B, T, 1024), np.float32)
    for b in range(B):
        acc = res[2 * b]["ot"].astype(np.float32) + res[2 * b + 1]["ot"].astype(
            np.float32
        )
        out[b] = acc.T + b_out[None, :]
    return out


# revision 13
# speedup vs baseline: 1.5117x; 1.0151x over previous
"""Causal self-attention (B=4, T=2048, D=1024, H=16) on 8 TRN2 NeuronCores.

Sharding: core i = (batch b = i//2, head-group g = i%2). Data parallel on B,
tensor parallel on heads (8 heads per group): qkv_proj columns and out_proj
rows split per head group. Each core computes a partial [D, T] output^T for
its batch; host sums the two group partials per batch, transposes, adds bias.

v3 design: all-bf16 PE datapath (fp32 PSUM accumulation), Q^T/K^T resident in
SBUF, and a qc-major fused schedule: attention blocks run (qc, pair) ordered so
each q-chunk level only needs transposes/projections for t <= 512*(qc+1);
setup work (x transpose, V/Q/K projections), the output projection and DMAs
are lazily interleaved between S/AV groups to keep the PE busy while the
scalar engine streams exp(). The 1/sqrt(dh) scale is folded into W_q on host.
Elementwise load is split: exp on Scalar, normalize/copies on Vector, causal
mask-mul + Q/K psum casts on GpSimd.
"""

import numpy as np
import ml_dtypes

import concourse.bacc as bacc
import concourse.tile as tile
import concourse.mybir as mybir
from concourse import bass_utils
from concourse.bass import ts

F32 = mybir.dt.float32
BF = mybir.dt.bfloat16
EXP = mybir.ActivationFunctionType.Exp

T = 2048
TT = 16          # t tiles of 128
NP = 4           # head pairs per core
NQC = 4          # q chunks of 512

_CACHE = {}
_last_in_maps = None


def _build(CT):
    """CT = number of 128-row c-tiles in the (possibly bias-augmented) x/W."""
    nc = bacc.Bacc("TRN2", target_bir_lowering=False, debug=False)
    C = CT * 128

    xa = nc.dram_tensor("xa", [T, C], BF, kind="ExternalInput").ap()
    wq = nc.dram_tensor("wq", [C, 512], BF, kind="ExternalInput").ap()
    wk = nc.dram_tensor("wk", [C, 512], BF, kind="ExternalInput").ap()
    wv = nc.dram_tensor("wv", [C, 512], BF, kind="ExternalInput").ap()
    wo = nc.dram_tensor("wo", [512, 1024], BF, kind="ExternalInput").ap()
    tri = nc.dram_tensor("tri", [128, 128], BF, kind="ExternalInput").ap()
    idn = nc.dram_tensor("idn", [128, 128], BF, kind="ExternalInput").ap()
    ot = nc.dram_tensor("ot", [1024, T], F32, kind="ExternalOutput").ap()

    with tile.TileContext(nc) as tc:
        with (
            tc.tile_pool(name="persist", bufs=1) as persist,
        ):
            mm = nc.tensor.matmul
            mmt = nc.tensor.transpose

            # resident tensors (per-partition bytes in comments)
            xT = persist.tile([128, CT, T], BF)            # 32K (CT=8)
            QK = persist.tile([128, NP, 2, NQC, 512], BF)  # 32K  [q=0/k=1]
            vS = persist.tile([128, TT, 8, 65], BF)        # 16.6K [k,tt,head,d+1]
            OT = persist.tile([128, NP, T], BF)            # 16.4K
            wq_sb = persist.tile([128, CT, NP, 128], BF)   # 8.2K
            wk_sb = persist.tile([128, CT, NP, 128], BF)   # 8.2K
            wv_sb = persist.tile([128, CT, 512], BF)       # 8.2K
            wo_sb = persist.tile([128, NP, 1024], BF)      # 8.2K
            tr = persist.tile([128, 128], BF)
            ident = persist.tile([128, 128], BF)
            nc.vector.memset(vS[:, :, :, 64:65], 1.0)
            nc.sync.dma_start(out=ident, in_=idn)

            with (
                tc.tile_pool(name="xnat", bufs=4) as xnat,
                tc.tile_pool(name="ptp", bufs=10) as ptpool,
                tc.tile_pool(name="rsm", bufs=6) as rpool,
                tc.tile_pool(name="rbcp", bufs=4) as rbcpool,
                tc.tile_pool(name="obnc", bufs=4) as opool,
                tc.tile_pool(name="psS", bufs=2, space="PSUM") as psS,      # 4 banks
                tc.tile_pool(name="psAv", bufs=2, space="PSUM") as psAv,    # 2 banks
                tc.tile_pool(name="pso", bufs=2, space="PSUM") as psO,      # 2 banks
            ):
                # ---------- setup work units (emitted lazily) ----------
                def load_weights():
                    nc.gpsimd.dma_start(out=tr, in_=tri)
                    nc.gpsimd.dma_start(
                        out=wq_sb,
                        in_=wq.rearrange("(ct P) (np f) -> P ct np f", P=128, np=NP),
                    )
                    nc.gpsimd.dma_start(
                        out=wk_sb,
                        in_=wk.rearrange("(ct P) (np f) -> P ct np f", P=128, np=NP),
                    )
                    nc.gpsimd.dma_start(
                        out=wv_sb,
                        in_=wv.rearrange("(ct P) f -> P ct f", P=128),
                    )

                def load_wo():
                    nc.gpsimd.dma_start(
                        out=wo_sb,
                        in_=wo.rearrange("(np P) f -> P np f", P=128),
                    )

                def transpose_tt(tt):
                    # one DMA per 128-token slab; transpose quads share a psum
                    # tile so one DVE copy moves 4 c-tiles into xT
                    xn = xnat.tile([128, C], BF)
                    nc.sync.dma_start(out=xn, in_=xa[ts(tt, 128), :])
                    for q4 in range(0, CT, 4):
                        ncc = min(4, CT - q4)
                        pt_ = psO.tile([128, 512], F32, name="pso", tag="pso")
                        for k in range(ncc):
                            cc = q4 + k
                            mmt(
                                pt_[:, 64 * k : 64 * k + 64].bitcast(BF),
                                xn[:, ts(cc, 128)],
                                ident,
                            )
                        nc.vector.tensor_copy(
                            out=xT[:, q4 : q4 + ncc, ts(tt, 128)],
                            in_=pt_[:, 0 : 64 * ncc]
                            .bitcast(BF)
                            .rearrange("p (c t) -> p c t", c=ncc),
                        )

                def vproj_tt(tt):
                    ps = psO.tile([128, 512], F32, name="pso", tag="pso")
                    for cc in range(CT):
                        mm(
                            ps,
                            lhsT=xT[:, cc, ts(tt, 128)],
                            rhs=wv_sb[:, cc, :],
                            start=(cc == 0),
                            stop=(cc == CT - 1),
                        )
                    nc.vector.tensor_copy(
                        out=vS[:, tt, :, 0:64],
                        in_=ps.rearrange("p (h d) -> p h d", h=8),
                    )

                def proj_chunk(p, kind, tc_):
                    # kind: 0 = q, 1 = k
                    w_sb = wq_sb if kind == 0 else wk_sb
                    ps = psO.tile([128, 512], F32, name="pso", tag="pso")
                    for cc in range(CT):
                        mm(
                            ps,
                            lhsT=w_sb[:, cc, p, :],
                            rhs=xT[:, cc, ts(tc_, 512)],
                            start=(cc == 0),
                            stop=(cc == CT - 1),
                        )
                    nc.vector.tensor_copy(out=QK[:, p, kind, tc_, :], in_=ps)

                def phase3_half(tc_, h):
                    for ft in range(4 * h, 4 * h + 4):
                        ps = psO.tile([128, 512], F32, name="pso", tag="pso")
                        for p in range(NP):
                            mm(
                                ps,
                                lhsT=wo_sb[:, p, ts(ft, 128)],
                                rhs=OT[:, p, ts(tc_, 512)],
                                start=(p == 0),
                                stop=(p == NP - 1),
                            )
                        ob = opool.tile([128, 512], F32)
                        if tc_ == NQC - 1:
                            # tail chunk: ACT queue is idle by now, DVE is not
                            nc.scalar.mul(out=ob, in_=ps, mul=1.0)
                        else:
                            nc.vector.tensor_copy(out=ob, in_=ps)
                        nc.sync.dma_start(out=ot[ts(ft, 128), ts(tc_, 512)], in_=ob)

                # lazy emission bookkeeping
                done_tp = [0]
                done_vp = [0]
                done_proj = set()
                fillers = []
                holdback = []

                def need_transp(up_to):
                    up_to = min(up_to, TT)
                    for tt in range(done_tp[0], up_to):
                        transpose_tt(tt)
                    done_tp[0] = max(done_tp[0], up_to)

                def need_vproj(up_to):
                    up_to = min(up_to, TT)
                    need_transp(up_to)
                    for tt in range(done_vp[0], up_to):
                        vproj_tt(tt)
                    done_vp[0] = max(done_vp[0], up_to)

                def need_tt(up_to):
                    need_transp(up_to)
                    need_vproj(up_to)

                def need_proj(p, kind, tc_):
                    if p >= NP or tc_ >= NQC:
                        return
                    key = (p, kind, tc_)
                    if key in done_proj:
                        return
                    done_proj.add(key)
                    proj_chunk(p, kind, tc_)

                def pop_filler():
                    if fillers:
                        fillers.pop(0)()

                # ---------- attention work ----------
                qch = lambda p, tc_: QK[:, p, 0, tc_, :]
                kch = lambda p, tc_: QK[:, p, 1, tc_, :]
                avs = {}
                pts = {}
                level_done = [0] * NQC

                def s_exp(p, qc, j):
                    off = max(0, 128 * j - 512 * qc)
                    sg = psS.tile([128, 2, 512], F32)
                    kc = kch(p, j // 4)
                    qc_t = qch(p, qc)
                    jo = 128 * (j % 4)
                    for m in range(2):
                        mm(
                            sg[:, m, off:],
                            lhsT=kc[64 * m : 64 * m + 64, jo : jo + 128],
                            rhs=qc_t[64 * m : 64 * m + 64, off:],
                            start=True,
                            stop=True,
                        )
                    ptile = ptpool.tile([128, 2, 512], BF)
                    nc.scalar.activation(
                        out=ptile[:, :, off:], in_=sg[:, :, off:], func=EXP
                    )
                    if j >= 4 * qc:
                        nc.vector.tensor_mul(
                            ptile[:, :, off : off + 128],
                            ptile[:, :, off : off + 128],
                            tr[:, None, :].to_broadcast([128, 2, 128]),
                        )
                    pts[(p, qc, j)] = (ptile, off)

                def av_mm(p, qc, j, nj):
                    ptile, off = pts.pop((p, qc, j))
                    av = avs[(p, qc)]
                    for m in range(2):
                        mm(
                            av[m][:65, off:],
                            lhsT=vS[:, j, 2 * p + m, :],
                            rhs=ptile[:, m, off:],
                            start=(j == 0),
                            stop=(j == nj - 1),
                        )

                def normalize(p, qc):
                    av = avs.pop((p, qc))
                    rsbs = []
                    for m in range(2):
                        rsb = rpool.tile([1, 512], F32, name="rsb", tag="rsb")
                        nc.vector.tensor_copy(out=rsb, in_=av[m][64:65, :])
                        # unnormalized O~ out of PSUM so the av bank frees fast
                        nc.vector.tensor_copy(
                            out=OT[64 * m : 64 * m + 64, p, ts(qc, 512)],
                            in_=av[m][0:64, :],
                        )
                        rsbs.append(rsb)
                    for m in range(2):
                        rinv = rpool.tile([1, 512], F32, name="rinv", tag="rinv")
                        nc.vector.reciprocal_approx_fast(out=rinv, in_=rsbs[m])
                        rinv_b = rpool.tile([1, 512], BF, name="rinvb", tag="rinvb")
                        nc.vector.tensor_copy(out=rinv_b, in_=rinv)
                        rb = rbcpool.tile([128, 512], BF, name="rb", tag="rb")
                        nc.gpsimd.partition_broadcast(rb, rinv_b)
                        sl = OT[64 * m : 64 * m + 64, p, ts(qc, 512)]
                        nc.vector.tensor_mul(sl, sl, rb[64 * m : 64 * m + 64, :])
                    level_done[qc] += 1
                    if level_done[qc] == NP:
                        # all pairs finished this t-chunk: output projection
                        fillers.insert(0, lambda a=qc: phase3_half(a, 0))
                        if qc == 2:
                            # hold half of chunk 2 back: fills the PE gap while
                            # the last block's softmax drains, keeping HAM warm
                            holdback.append(lambda a=qc: phase3_half(a, 1))
                        else:
                            fillers.insert(1, lambda a=qc: phase3_half(a, 1))

                # groups: qc-major so each level only needs setup through
                # t = 512*(qc+1); pairs within a level share the causal shape
                groups = []
                for qc in range(NQC):
                    for p in range(NP):
                        nj = 4 * qc + 4
                        js = list(range(nj))
                        sub = [js[i : i + 3] for i in range(0, nj, 3)]
                        for gi, jg in enumerate(sub):
                            groups.append(
                                (p, qc, nj, jg, gi == 0, gi == len(sub) - 1)
                            )

                def av_group(gi):
                    p, qc, nj, jg, first, last = groups[gi]
                    if first:
                        avs[(p, qc)] = [
                            psAv.tile([128, 512], F32, name="av", tag="av")
                            for _ in range(2)
                        ]
                    for j in jg:
                        av_mm(p, qc, j, nj)
                    if last:
                        normalize(p, qc)

                # prologue: weights first (gpsimd queue, parallel to x slabs)
                load_weights()
                load_wo()
                need_transp(4)
                need_proj(0, 1, 0)
                need_proj(0, 0, 0)
                fillers.append(lambda: need_vproj(4))

                LAG = 2
                for i in range(len(groups) + LAG):
                    if i < len(groups):
                        p, qc, nj, jg, first, last = groups[i]
                        if first:
                            # hard deps for this block (usually no-ops)
                            need_transp(4 * qc + 4)
                            need_proj(p, 1, qc)
                            need_proj(p, 0, qc)
                            fillers.insert(0, lambda u=4 * qc + 4: need_vproj(u))
                            # soft prefetches, spread between groups
                            if p == 0:
                                fillers.append(lambda u=4 * qc + 8: need_tt(u))
                            fillers.append(
                                lambda b=p, a=qc + 1: (
                                    need_proj(b, 1, a),
                                    need_proj(b, 0, a),
                                )
                            )
                        for j in jg:
                            s_exp(p, qc, j)
                        pop_filler()
                    if i >= LAG and (i - LAG) % 2 == 1:
                        av_group(i - LAG - 1)
                        av_group(i - LAG)
                        pop_filler()
                if len(groups) % 2 == 1:
                    av_group(len(groups) - 1)
                for f in holdback:
                    f()
                while fillers:
                    pop_filler()

    nc.compile()
    return nc


def kernel(x, W_qkv, b_qkv, W_out, b_out):
    global _last_in_maps
    x = np.asarray(x, dtype=np.float32)
    W_qkv = np.asarray(W_qkv, dtype=np.float32)
    b_qkv = np.asarray(b_qkv, dtype=np.float32)
    W_out = np.asarray(W_out, dtype=np.float32)
    b_out = np.asarray(b_out, dtype=np.float32)
    B = x.shape[0]
    BF_NP = ml_dtypes.bfloat16

    aug = bool(np.any(b_qkv))
    CT = 9 if aug else 8
    if CT not in _CACHE:
        _CACHE[CT] = _build(CT)
    nc = _CACHE[CT]

    # triangle keep-mask for the diagonal 128 block: [p, c] = 1 if c >= p
    tri = (np.arange(128)[None, :] >= np.arange(128)[:, None]).astype(BF_NP)
    SCALE = 1.0 / np.sqrt(64.0)

    in_maps = []
    for core in range(8):
        b, g = core // 2, core % 2
        xa = x[b]
        if aug:
            pad = np.zeros((T, 128), np.float32)
            pad[:, 0] = 1.0
            xa = np.concatenate([xa, pad], axis=1)

        def wslice(col0, scale=1.0):
            w = W_qkv[:, col0 + 512 * g : col0 + 512 * g + 512]
            if aug:
                extra = np.zeros((128, 512), np.float32)
                extra[0] = b_qkv[col0 + 512 * g : col0 + 512 * g + 512]
                w = np.concatenate([w, extra], axis=0)
            return np.ascontiguousarray(w * scale).astype(BF_NP)

        in_maps.append(
            {
                "xa": np.ascontiguousarray(xa).astype(BF_NP),
                "wq": wslice(0, SCALE),
                "wk": wslice(1024),
                "wv": wslice(2048),
                "wo": np.ascontiguousarray(
                    W_out[512 * g : 512 * g + 512, :]
                ).astype(BF_NP),
                "tri": tri,
                "idn": np.eye(128, dtype=BF_NP),
            }
        )

    _last_in_maps = in_maps
    res = bass_utils.run_bass_kernel_spmd(nc, in_maps, list(range(8))).results
    out = np.empty((B, T, 1024), np.float32)
    for b in range(B):
        acc = res[2 * b]["ot"].astype(np.float32) + res[2 * b + 1]["ot"].astype(
            np.float32
        )
        out[b] = acc.T + b_out[None, :]
    return out
